# revision 2
# baseline (speedup 1.0000x reference)
"""Trainium2 Bass kernel for nn_MultiHeadAttention_48395691492077.

Reference (B=4, S=2048, D=1024, single head, anti-causal triu mask):
    qkv = x @ wqkv; q,k,v = split(qkv)
    scores = triu(q @ k^T / sqrt(B));  masked softmax over keys t >= s
    x2  = softmax(scores) @ v @ w_lin + b_lin + x
    out = relu(x2 @ w_ff1 + b_ff1) @ w_ff2 + b_ff2 + x2

Sharding: 8 cores = 4 batches x 2 query-halves. Each core computes
attention + MLP for its own 1024 queries against the full 2048-key
sequence of its batch. The program is identical on all cores (SPMD);
per-core differences (which queries, which mask pattern) are carried in
the input data plus one branch on the query-parity register.

Device algebra (transposed; no on-chip transposes, no K/V projections):
    uT = wzq^T.T @ qxT            with wzq = (Wq @ Wk^T)/2  (host-fused)
    scoresT[t,s] = sum_d xT[d,t] * uT[d,s]     (keys are raw x!)
    expT = exp(scoresT) * mask01               (no max-subtraction)
    den[s] broadcast = ones[128,128].T @ expT (PE), rbs = 1/den (DVE)
    H^T[d,s] = x[t,d].T @ expT  (A@X; V projection deferred)
    attnT = H^T * rbs
    x2T = wvl.T @ attnT + (xT + b_lin)  with wvl = Wv @ w_lin (host-fused:
          A@(X@Wv)@w_lin == (A@X)@(Wv@w_lin) by associativity)
    hT = relu(w_ff1.T @ x2T + b_ff1)
    outT = w_ff2.T @ hT + x2T               (+ b_ff2 added on host)
Matmul inputs are bf16 (fp32 PSUM accumulation); residuals are fp32.
"""

import numpy as np
import ml_dtypes

B, S, D = 4, 2048, 1024
NCORES = 8
BF16 = ml_dtypes.bfloat16

NT = S // 128            # 16 t-chunks
ND = D // 128            # 8 chunks of 128 along any D-sized dim

# global query-column starts of (sb0, sb1) per parity
SB_GLOBAL = {0: (0, 1536), 1: (512, 1024)}
# t-chunks each (parity, s-block) actually needs (branch-specialized)
SB_SLOTS = {
    0: {0: list(range(0, NT)), 1: list(range(12, NT))},
    1: {0: list(range(4, NT)), 1: list(range(8, NT))},
}


_COMPILED = None
_LAST_IN_MAPS = None


def _mask_order(parity: int):
    return [(sb, tc) for sb in (0, 1) for tc in SB_SLOTS[parity][sb]]


def _build_masks(parity: int) -> np.ndarray:
    """[20, 128, 512] bf16 multiplicative masks, one per processed block."""
    order = _mask_order(parity)
    m = np.zeros((len(order), 128, 512), np.float32)
    ii = np.arange(128)[:, None]
    jj = np.arange(512)[None, :]
    for k, (sb, tc) in enumerate(order):
        s0 = SB_GLOBAL[parity][sb]
        m[k] = ((128 * tc + ii) >= (s0 + jj)).astype(np.float32)
    return m.astype(BF16)


def _build_program():
    from contextlib import ExitStack
    import concourse.bacc as bacc
    import concourse.mybir as mybir
    import concourse.tile as tile

    f32 = mybir.dt.float32
    b16 = mybir.dt.bfloat16
    AF = mybir.ActivationFunctionType

    nc = bacc.Bacc("TRN2", target_bir_lowering=False, debug=False,
                   num_devices=NCORES)

    xT_d = nc.dram_tensor("xT", [D, S], b16, kind="ExternalInput")
    xn_d = nc.dram_tensor("xn", [S, D], b16, kind="ExternalInput")
    qxT_d = nc.dram_tensor("qxT", [D, 1024], b16, kind="ExternalInput")
    xq_d = nc.dram_tensor("xq", [D, 1024], f32, kind="ExternalInput")
    wzq_d = nc.dram_tensor("wzq", [D, D], b16, kind="ExternalInput")
    wvl_d = nc.dram_tensor("wvl", [D, D], b16, kind="ExternalInput")
    wff1_d = nc.dram_tensor("wff1", [D, D], b16, kind="ExternalInput")
    wff2_d = nc.dram_tensor("wff2", [D, D], b16, kind="ExternalInput")
    masks_d = nc.dram_tensor("masks", [20, 128, 512], b16, kind="ExternalInput")
    par_d = nc.dram_tensor("par", [1, 1], mybir.dt.uint32, kind="ExternalInput")
    bf1_d = nc.dram_tensor("bf1", [ND, 128], f32, kind="ExternalInput")
    outT_d = nc.dram_tensor("outT", [D, 1024], f32, kind="ExternalOutput")

    with tile.TileContext(nc) as tc:
        es = ExitStack()
        with es:
            pp = es.enter_context(tc.tile_pool(name="persist", bufs=1))
            sp = es.enter_context(tc.tile_pool(name="stream", bufs=2))
            ps = es.enter_context(
                tc.tile_pool(name="ps", bufs=8, space="PSUM"))
            esB = es.enter_context(ExitStack())
            pb = esB.enter_context(tc.tile_pool(name="pB", bufs=1))
            esA = ExitStack()
            pa = esA.enter_context(tc.tile_pool(name="pA", bufs=1,
                                                side="right"))

            def psum():
                t = ps.tile([128, 512], f32, tag="mm", bufs=8, name="mmps")
                return t

            # ---- constants ----
            ones_sq = pp.tile([128, 128], b16, tag="ones_sq", bufs=1)
            nc.vector.memset(ones_sq[:], 1.0)
            # warm the PE HAM clock-gate while input DMAs are in flight
            wups = psum()
            for i in range(64):
                nc.tensor.matmul(wups[:, 0:128], ones_sq[:], ones_sq[:],
                                 start=(i == 0), stop=(i == 63))

            # ---- input loads ----
            def chunked(dram, cols):
                return dram.ap().rearrange("(c p) n -> p c n", p=128)

            # wzq + qx gate the very first u matmuls: finest chunks first
            wzq_a = pa.tile([128, ND, D], b16, tag="wzq", bufs=1)
            for c0, c1 in ((0, 128), (128, 512), (512, 1024)):
                nc.sync.dma_start(wzq_a[:, :, c0:c1],
                                  wzq_d.ap()[:, c0:c1]
                                  .rearrange("(c p) n -> p c n", p=128))
            qx_a = pa.tile([128, ND, 1024], b16, tag="qx", bufs=1)
            for c0, c1 in ((0, 512), (512, 1024)):
                nc.sync.dma_start(qx_a[:, :, c0:c1],
                                  qxT_d.ap()[:, c0:c1]
                                  .rearrange("(c p) n -> p c n", p=128))
            # xT feeds the scores pass (starts after u-sb0, ~25us in)
            xt_a = pb.tile([128, ND, S], b16, tag="xt", bufs=1)
            xt_splits = [0, 512, 1024, 1536, 2048]
            for c0, c1 in zip(xt_splits, xt_splits[1:]):
                nc.sync.dma_start(
                    xt_a[:, :, c0:c1],
                    xT_d.ap()[:, c0:c1]
                    .rearrange("(c p) n -> p c n", p=128))
            # x natural layout [t, d] feeds the A@X pass (later still)
            xn = [pb.tile([128, D], b16, tag=f"xn{t}", bufs=1, name=f"xn{t}")
                  for t in range(NT)]
            for t in range(NT):
                nc.sync.dma_start(xn[t][:], xn_d.ap()[t * 128:(t + 1) * 128, :])
            # b_ff1 laid out [128, ND]: bias column fc serves f-chunk fc
            bf1_t = pp.tile([128, ND], f32, tag="bf1", bufs=1)
            nc.sync.dma_start(bf1_t[:], bf1_d.ap().rearrange("c p -> p c"))
            wzq_t = [wzq_a[:, d] for d in range(ND)]
            xt = [xt_a[:, d] for d in range(ND)]
            qx = [qx_a[:, d] for d in range(ND)]

            # ---- phase A: uT[d, s] = sum_a wzq[a,d] * qxT[a,s] ----
            ut = [pb.tile([128, 1024], b16, tag=f"ut{m}", bufs=1,
                          name=f"ut{m}") for m in range(ND)]
            for m in range(ND):
                ups = [psum() for _ in range(2)]
                for a in range(ND):
                    for sb in range(2):
                        nc.tensor.matmul(
                            ups[sb][:],
                            wzq_t[a][:, m * 128:(m + 1) * 128],
                            qx[a][:, sb * 512:(sb + 1) * 512],
                            start=(a == 0), stop=(a == ND - 1))
                for sb in range(2):
                    nc.vector.tensor_copy(
                        ut[m][:, sb * 512:(sb + 1) * 512], ups[sb][:])

            # ---- free phase-A inputs; right pool for attn + phase-C weights
            esA.close()
            pr = es.enter_context(tc.tile_pool(name="pAC", bufs=1,
                                               side="right"))
            wl_a = pr.tile([128, ND, D], b16, tag="wl", bufs=1)
            nc.sync.dma_start(wl_a[:], chunked(wvl_d, D))
            wf1_a = pr.tile([128, ND, D], b16, tag="wf1", bufs=1)
            nc.sync.dma_start(wf1_a[:], chunked(wff1_d, D))
            wf2_a = pr.tile([128, ND, D], b16, tag="wf2", bufs=1)
            nc.sync.dma_start(wf2_a[:], chunked(wff2_d, D))
            wvl_t = [wl_a[:, d] for d in range(ND)]
            wff1_t = [wf1_a[:, d] for d in range(ND)]
            wff2_t = [wf2_a[:, d] for d in range(ND)]

            attn = [pr.tile([128, 1024], b16, tag=f"at{d}", bufs=1,
                            name=f"at{d}") for d in range(ND)]

            def phase_b(parity):
                sb_slots = SB_SLOTS[parity]
                order = _mask_order(parity)
                # pass 1: scoresT -> exp -> mask, tc-outer
                et = {}
                for tcn in range(NT):
                    sbs = [sb for sb in (0, 1) if tcn in sb_slots[sb]]
                    scp = {sb: psum() for sb in sbs}
                    for d in range(ND):
                        for sb in sbs:
                            nc.tensor.matmul(
                                scp[sb][:],
                                xt[d][:, tcn * 128:(tcn + 1) * 128],
                                ut[d][:, sb * 512:(sb + 1) * 512],
                                start=(d == 0), stop=(d == ND - 1))
                    for sb in sbs:
                        e = pb.tile([128, 512], b16, tag=f"et{sb}_{tcn}",
                                    bufs=1, name=f"et{parity}_{sb}_{tcn}")
                        et[(sb, tcn)] = e
                        nc.scalar.activation(e[:], scp[sb][:], AF.Exp)
                        kidx = order.index((sb, tcn))
                        mk = sp.tile([128, 512], b16, tag="mks", bufs=6,
                                     name=f"mk{parity}_{kidx}")
                        nc.sync.dma_start(mk[:], masks_d.ap()[kidx])
                        nc.vector.tensor_mul(e[:], e[:], mk[:])

                # pass 2: den (broadcast), recip, A@X, normalize
                rbs = {}
                for sb in (0, 1):
                    slots = sb_slots[sb]
                    den_ps = psum()
                    for k, tcn in enumerate(slots):
                        nc.tensor.matmul(
                            den_ps[:], ones_sq[:], et[(sb, tcn)][:],
                            start=(k == 0), stop=(k == len(slots) - 1))
                    r = sp.tile([128, 512], f32, tag="rbs", bufs=2,
                                name=f"rbs{parity}_{sb}")
                    nc.vector.reciprocal(r[:], den_ps[:])
                    rbs[sb] = r

                for dc in range(ND):
                    axp = {sb: psum() for sb in (0, 1)}
                    for tcn in range(NT):
                        for sb in (0, 1):
                            slots = sb_slots[sb]
                            if tcn not in slots:
                                continue
                            nc.tensor.matmul(
                                axp[sb][:],
                                xn[tcn][:, dc * 128:(dc + 1) * 128],
                                et[(sb, tcn)][:],
                                start=(tcn == slots[0]),
                                stop=(tcn == slots[-1]))
                    for sb in (0, 1):
                        nc.vector.tensor_mul(
                            attn[dc][:, sb * 512:(sb + 1) * 512],
                            axp[sb][:], rbs[sb][:])

            par_regs = nc.alloc_registers("par_regs")
            nc.regs_load(par_regs, par_d.ap()[0:1, 0:1])
            par = nc.snap(par_regs, donate=True, min_val=0, max_val=1)
            with tc.If(par < 1) as cmp:
                phase_b(0)
            with cmp.Else():
                phase_b(1)

            # ---- free pB (ut/xt/xn/et); left pool for phase-C tiles ----
            esB.close()
            esC = es.enter_context(ExitStack())
            pc = esC.enter_context(tc.tile_pool(name="pC", bufs=1))

            x2f = [pc.tile([128, 1024], f32, tag=f"x2f{d}", bufs=1,
                           name=f"x2f{d}") for d in range(ND)]
            x2b = [pc.tile([128, 1024], b16, tag=f"x2b{d}", bufs=1,
                           name=f"x2b{d}") for d in range(ND)]
            ht = [pc.tile([128, 1024], b16, tag=f"ht{d}", bufs=1,
                          name=f"ht{d}") for d in range(ND)]

            for oc in range(ND):
                for s2 in range(2):
                    cps = psum()
                    for d in range(ND):
                        nc.tensor.matmul(
                            cps[:],
                            wvl_t[d][:, oc * 128:(oc + 1) * 128],
                            attn[d][:, s2 * 512:(s2 + 1) * 512],
                            start=(d == 0), stop=(d == ND - 1))
                    xqt = sp.tile([128, 512], f32, tag="xqt", bufs=4,
                                  name=f"xqt{oc}_{s2}")
                    nc.sync.dma_start(
                        xqt[:],
                        xq_d.ap()[oc * 128:(oc + 1) * 128,
                                  s2 * 512:(s2 + 1) * 512])
                    cc = slice(s2 * 512, (s2 + 1) * 512)
                    nc.vector.tensor_add(x2f[oc][:, cc], cps[:], xqt[:])
                    nc.vector.tensor_copy(x2b[oc][:, cc], x2f[oc][:, cc])

            for fc in range(ND):
                for s2 in range(2):
                    cps = psum()
                    for d in range(ND):
                        nc.tensor.matmul(
                            cps[:],
                            wff1_t[d][:, fc * 128:(fc + 1) * 128],
                            x2b[d][:, s2 * 512:(s2 + 1) * 512],
                            start=(d == 0), stop=(d == ND - 1))
                    cc = slice(s2 * 512, (s2 + 1) * 512)
                    nc.scalar.activation(ht[fc][:, cc], cps[:], AF.Relu,
                                         bias=bf1_t[:, fc:fc + 1])

            for oc in range(ND):
                for s2 in range(2):
                    cps = psum()
                    for f in range(ND):
                        nc.tensor.matmul(
                            cps[:],
                            wff2_t[f][:, oc * 128:(oc + 1) * 128],
                            ht[f][:, s2 * 512:(s2 + 1) * 512],
                            start=(f == 0), stop=(f == ND - 1))
                    cc = slice(s2 * 512, (s2 + 1) * 512)
                    ot = sp.tile([128, 512], f32, tag="ot", bufs=4,
                                 name=f"ot{oc}_{s2}")
                    nc.vector.tensor_add(ot[:], cps[:], x2f[oc][:, cc])
                    nc.sync.dma_start(
                        outT_d.ap()[oc * 128:(oc + 1) * 128, cc], ot[:])

    nc.compile()
    return nc


def _get_program():
    global _COMPILED
    if _COMPILED is None:
        _COMPILED = _build_program()
    return _COMPILED


def kernel(x, wqkv, w_lin, b_lin, w_ff1, b_ff1, w_ff2, b_ff2):
    from concourse.bass_utils import run_bass_kernel_spmd

    x = np.asarray(x, np.float32)
    wqkv = np.asarray(wqkv, np.float32)
    Wq = wqkv[:, :D].astype(np.float64)
    Wk = wqkv[:, D:2 * D].astype(np.float64)
    Wv = wqkv[:, 2 * D:].astype(np.float64)

    wzq = ((Wq @ Wk.T) / 2.0).astype(BF16)      # [a, d] natural layout
    wvl = (Wv @ np.asarray(w_lin, np.float64)).astype(BF16)  # Wv @ w_lin
    wff1 = np.asarray(w_ff1, np.float32).astype(BF16)
    wff2 = np.asarray(w_ff2, np.float32).astype(BF16)
    masks = {p: _build_masks(p) for p in (0, 1)}

    in_maps = []
    qcols_by_parity = {
        0: np.r_[0:512, 1536:2048],
        1: np.r_[512:1536],
    }
    b_lin = np.asarray(b_lin, np.float32)
    b_ff1 = np.asarray(b_ff1, np.float32)
    b_ff2 = np.asarray(b_ff2, np.float32)
    bf1 = np.ascontiguousarray(b_ff1.reshape(ND, 128))
    for c in range(NCORES):
        b, h = c // 2, c % 2
        xT32 = np.ascontiguousarray(x[b].T)               # [D, S] f32
        qcols = qcols_by_parity[h]
        qxT32 = np.ascontiguousarray(xT32[:, qcols])      # [D, 1024]
        in_maps.append({
            "xT": xT32.astype(BF16),
            "xn": x[b].astype(BF16),                      # [S, D] natural
            "qxT": qxT32.astype(BF16),
            "xq": qxT32 + b_lin[:, None],                 # b_lin folded in
            "wzq": wzq,
            "wvl": wvl,
            "wff1": wff1,
            "wff2": wff2,
            "masks": masks[h],
            "bf1": bf1,
            "par": np.full((1, 1), h, np.uint32),
        })

    global _LAST_IN_MAPS
    _LAST_IN_MAPS = in_maps
    nc = _get_program()
    res = run_bass_kernel_spmd(nc, in_maps, core_ids=list(range(NCORES)))

    out = np.empty((B, S, D), np.float32)
    for c in range(NCORES):
        b, h = c // 2, c % 2
        ol = res.results[c]["outT"].T                     # [1024 s, D]
        if h == 0:
            out[b, 0:512] = ol[:512]
            out[b, 1536:2048] = ol[512:]
        else:
            out[b, 512:1536] = ol
    out += b_ff2[None, None, :]
    return out


# revision 3
# speedup vs baseline: 1.0144x; 1.0144x over previous
"""Trainium2 Bass kernel for nn_MultiHeadAttention_48395691492077.

Reference (B=4, S=2048, D=1024, single head, anti-causal triu mask):
    qkv = x @ wqkv; q,k,v = split(qkv)
    scores = triu(q @ k^T / sqrt(B));  masked softmax over keys t >= s
    x2  = softmax(scores) @ v @ w_lin + b_lin + x
    out = relu(x2 @ w_ff1 + b_ff1) @ w_ff2 + b_ff2 + x2

Sharding: 8 cores = 4 batches x 2 query-halves. Each core computes
attention + MLP for its own 1024 queries against the full 2048-key
sequence of its batch. The program is identical on all cores (SPMD);
per-core differences (which queries, which mask pattern) are carried in
the input data plus one branch on the query-parity register.

Device algebra (transposed; no on-chip transposes, no K/V projections):
    uT = wzq^T.T @ qxT            with wzq = (Wq @ Wk^T)/2  (host-fused)
    scoresT[t,s] = sum_d xT[d,t] * uT[d,s]     (keys are raw x!)
    expT = exp(scoresT); diagonal 128-strips *= lower-tri mask
    den[s] broadcast = ones[128,128].T @ expT (PE), rbs = 1/den (DVE)
    H^T[d,s] = x[t,d].T @ expT  (A@X; V projection deferred)
    attnT = H^T * rbs
    x2T = wvl.T @ attnT + (xT + b_lin)  with wvl = Wv @ w_lin (host-fused:
          A@(X@Wv)@w_lin == (A@X)@(Wv@w_lin) by associativity)
    hT = relu(w_ff1.T @ x2T + b_ff1)
    outT = w_ff2.T @ hT + x2T               (+ b_ff2 added on host)
Blocks crossing the anti-causal diagonal use exact matmul widths
(128/256/384) instead of full 512; PSUM accumulation runs widest-first
so every column is initialized by the start=True matmul.
Matmul inputs are bf16 (fp32 PSUM accumulation); residuals are fp32.
"""

import numpy as np
import ml_dtypes

B, S, D = 4, 2048, 1024
NCORES = 8
BF16 = ml_dtypes.bfloat16

NT = S // 128            # 16 t-chunks
ND = D // 128            # 8 chunks of 128 along any D-sized dim

# global query-column starts of (sb0, sb1) per parity
SB_GLOBAL = {0: (0, 1536), 1: (512, 1024)}
# t-chunks each (parity, s-block) actually needs (branch-specialized)
SB_SLOTS = {
    p: {sb: list(range(SB_GLOBAL[p][sb] // 128, NT)) for sb in (0, 1)}
    for p in (0, 1)
}


def _width(parity, sb, tc):
    """Valid column count of block (sb, tc): cols [0, w) of the 512."""
    return min(512, 128 * tc - SB_GLOBAL[parity][sb] + 128)


def _is_diag(parity, sb, tc):
    """Block whose last 128 columns lie on the anti-causal diagonal."""
    return tc - SB_GLOBAL[parity][sb] // 128 < 4


_COMPILED = None
_LAST_IN_MAPS = None


def _build_program():
    from contextlib import ExitStack
    import concourse.bacc as bacc
    import concourse.mybir as mybir
    import concourse.tile as tile

    f32 = mybir.dt.float32
    b16 = mybir.dt.bfloat16
    AF = mybir.ActivationFunctionType

    nc = bacc.Bacc("TRN2", target_bir_lowering=False, debug=False,
                   num_devices=NCORES)

    xT_d = nc.dram_tensor("xT", [D, S], b16, kind="ExternalInput")
    xn_d = nc.dram_tensor("xn", [S, D], b16, kind="ExternalInput")
    qxT_d = nc.dram_tensor("qxT", [D, 1024], b16, kind="ExternalInput")
    xq_d = nc.dram_tensor("xq", [D, 1024], f32, kind="ExternalInput")
    wzq_d = nc.dram_tensor("wzq", [D, D], b16, kind="ExternalInput")
    wvl_d = nc.dram_tensor("wvl", [D, D], b16, kind="ExternalInput")
    wff1_d = nc.dram_tensor("wff1", [D, D], b16, kind="ExternalInput")
    wff2_d = nc.dram_tensor("wff2", [D, D], b16, kind="ExternalInput")
    tri_d = nc.dram_tensor("tri", [128, 128], b16, kind="ExternalInput")
    par_d = nc.dram_tensor("par", [1, 1], mybir.dt.uint32, kind="ExternalInput")
    bf1_d = nc.dram_tensor("bf1", [ND, 128], f32, kind="ExternalInput")
    outT_d = nc.dram_tensor("outT", [D, 1024], f32, kind="ExternalOutput")

    with tile.TileContext(nc) as tc:
        es = ExitStack()
        with es:
            pp = es.enter_context(tc.tile_pool(name="persist", bufs=1))
            sp = es.enter_context(tc.tile_pool(name="stream", bufs=2))
            ps = es.enter_context(
                tc.tile_pool(name="ps", bufs=8, space="PSUM"))
            esB = es.enter_context(ExitStack())
            pb = esB.enter_context(tc.tile_pool(name="pB", bufs=1))
            esA = ExitStack()
            pa = esA.enter_context(tc.tile_pool(name="pA", bufs=1,
                                                side="right"))

            def psum():
                t = ps.tile([128, 512], f32, tag="mm", bufs=8, name="mmps")
                return t

            # ---- constants ----
            ones_sq = pp.tile([128, 128], b16, tag="ones_sq", bufs=1)
            nc.vector.memset(ones_sq[:], 1.0)
            tri_t = pp.tile([128, 128], b16, tag="tri", bufs=1)
            # warm the PE HAM clock-gate while the first input DMAs land
            wups = psum()
            for i in range(12):
                nc.tensor.matmul(wups[:, 0:128], ones_sq[:], ones_sq[:],
                                 start=(i == 0), stop=(i == 11))

            # ---- input loads (arrival-ordered for phase-A pipelining) ----
            def rows(dram, r0, r1, c0, c1):
                return dram.ap()[r0:r1, c0:c1].rearrange(
                    "(c p) n -> p c n", p=128)

            wzq_a = pa.tile([128, ND, D], b16, tag="wzq", bufs=1)
            qx_a = pa.tile([128, ND, 1024], b16, tag="qx", bufs=1)
            nc.sync.dma_start(wzq_a[:, 0:4, 0:512], rows(wzq_d, 0, 512, 0, 512))
            nc.sync.dma_start(qx_a[:, 0:4, :], rows(qxT_d, 0, 512, 0, 1024))
            nc.sync.dma_start(wzq_a[:, 4:8, 0:512],
                              rows(wzq_d, 512, 1024, 0, 512))
            nc.sync.dma_start(qx_a[:, 4:8, :], rows(qxT_d, 512, 1024, 0, 1024))
            nc.sync.dma_start(wzq_a[:, :, 512:1024],
                              rows(wzq_d, 0, 1024, 512, 1024))
            # xT feeds the scores pass; column chunks in tc order
            xt_a = pb.tile([128, ND, S], b16, tag="xt", bufs=1)
            for c0, c1 in ((0, 512), (512, 1024), (1024, 1536), (1536, 2048)):
                nc.sync.dma_start(xt_a[:, :, c0:c1], rows(xT_d, 0, D, c0, c1))
            # x natural layout [t, d] feeds the A@X pass (later still)
            xn_a = pb.tile([128, NT, D], b16, tag="xn", bufs=1)
            nc.sync.dma_start(xn_a[:], rows(xn_d, 0, S, 0, D))
            nc.sync.dma_start(tri_t[:], tri_d.ap())
            # b_ff1 laid out [128, ND]: bias column fc serves f-chunk fc
            bf1_t = pp.tile([128, ND], f32, tag="bf1", bufs=1)
            nc.sync.dma_start(bf1_t[:], bf1_d.ap().rearrange("c p -> p c"))
            wzq_t = [wzq_a[:, d] for d in range(ND)]
            xt = [xt_a[:, d] for d in range(ND)]
            xn = [xn_a[:, t] for t in range(NT)]
            qx = [qx_a[:, d] for d in range(ND)]

            # ---- phase A: uT[d, s] = sum_a wzq[a,d] * qxT[a,s] ----
            # a-outer in two m-halves (8 PSUM banks each) so compute starts
            # as soon as the first a-chunks of wzq/qx land.
            ut = [pb.tile([128, 1024], b16, tag=f"ut{m}", bufs=1,
                          name=f"ut{m}") for m in range(ND)]
            for half in range(2):
                ms = range(half * 4, half * 4 + 4)
                ups = {(m, sb): psum() for m in ms for sb in range(2)}
                for a in range(ND):
                    for m in ms:
                        for sb in range(2):
                            nc.tensor.matmul(
                                ups[(m, sb)][:],
                                wzq_t[a][:, m * 128:(m + 1) * 128],
                                qx[a][:, sb * 512:(sb + 1) * 512],
                                start=(a == 0), stop=(a == ND - 1))
                for m in ms:
                    for sb in range(2):
                        nc.vector.tensor_copy(
                            ut[m][:, sb * 512:(sb + 1) * 512],
                            ups[(m, sb)][:])

            # ---- free phase-A inputs; right pool for attn + phase-C weights
            esA.close()
            pr = es.enter_context(tc.tile_pool(name="pAC", bufs=1,
                                               side="right"))
            wl_a = pr.tile([128, ND, D], b16, tag="wl", bufs=1)
            nc.sync.dma_start(wl_a[:], rows(wvl_d, 0, D, 0, D))
            wf1_a = pr.tile([128, ND, D], b16, tag="wf1", bufs=1)
            nc.sync.dma_start(wf1_a[:], rows(wff1_d, 0, D, 0, D))
            wf2_a = pr.tile([128, ND, D], b16, tag="wf2", bufs=1)
            nc.sync.dma_start(wf2_a[:], rows(wff2_d, 0, D, 0, D))
            wvl_t = [wl_a[:, d] for d in range(ND)]
            wff1_t = [wf1_a[:, d] for d in range(ND)]
            wff2_t = [wf2_a[:, d] for d in range(ND)]

            attn = [pr.tile([128, 1024], b16, tag=f"at{d}", bufs=1,
                            name=f"at{d}") for d in range(ND)]

            def phase_b(parity):
                sb_slots = SB_SLOTS[parity]
                # pass 1: scoresT -> exp -> diag mask, tc-outer
                et = {}
                for tcn in range(NT):
                    work = [(sb, _width(parity, sb, tcn))
                            for sb in (0, 1) if tcn in sb_slots[sb]]
                    scp = {sb: psum() for sb, _ in work}
                    for d in range(ND):
                        for sb, w in work:
                            nc.tensor.matmul(
                                scp[sb][:, 0:w],
                                xt[d][:, tcn * 128:(tcn + 1) * 128],
                                ut[d][:, sb * 512:sb * 512 + w],
                                start=(d == 0), stop=(d == ND - 1))
                    for sb, w in work:
                        e = pb.tile([128, w], b16, tag=f"et{sb}_{tcn}",
                                    bufs=1, name=f"et{parity}_{sb}_{tcn}")
                        et[(sb, tcn)] = e
                        nc.scalar.activation(e[:], scp[sb][:, 0:w], AF.Exp)
                        if _is_diag(parity, sb, tcn):
                            nc.vector.tensor_mul(
                                e[:, w - 128:w], e[:, w - 128:w], tri_t[:])

                # pass 2: den (broadcast), recip, A@X, normalize.
                # Accumulate widest-first (descending tc) so the start=True
                # matmul initializes the full 512 columns.
                rbs = {}
                for sb in (0, 1):
                    slots = sb_slots[sb][::-1]
                    den_ps = psum()
                    for k, tcn in enumerate(slots):
                        w = _width(parity, sb, tcn)
                        nc.tensor.matmul(
                            den_ps[:, 0:w], ones_sq[:], et[(sb, tcn)][:],
                            start=(k == 0), stop=(k == len(slots) - 1))
                    r = sp.tile([128, 512], f32, tag="rbs", bufs=2,
                                name=f"rbs{parity}_{sb}")
                    nc.vector.reciprocal(r[:], den_ps[:])
                    rbs[sb] = r

                for dc in range(ND):
                    axp = {sb: psum() for sb in (0, 1)}
                    for sb in (0, 1):
                        slots = sb_slots[sb][::-1]
                        for k, tcn in enumerate(slots):
                            w = _width(parity, sb, tcn)
                            nc.tensor.matmul(
                                axp[sb][:, 0:w],
                                xn[tcn][:, dc * 128:(dc + 1) * 128],
                                et[(sb, tcn)][:],
                                start=(k == 0),
                                stop=(k == len(slots) - 1))
                    for sb in (0, 1):
                        nc.vector.tensor_mul(
                            attn[dc][:, sb * 512:(sb + 1) * 512],
                            axp[sb][:], rbs[sb][:])

            par_regs = nc.alloc_registers("par_regs")
            nc.regs_load(par_regs, par_d.ap()[0:1, 0:1])
            par = nc.snap(par_regs, donate=True, min_val=0, max_val=1)
            with tc.If(par < 1) as cmp:
                phase_b(0)
            with cmp.Else():
                phase_b(1)

            # ---- free pB (ut/xt/xn/et); left pool for phase-C tiles ----
            esB.close()
            esC = es.enter_context(ExitStack())
            pc = esC.enter_context(tc.tile_pool(name="pC", bufs=1))

            x2f = [pc.tile([128, 1024], f32, tag=f"x2f{d}", bufs=1,
                           name=f"x2f{d}") for d in range(ND)]
            x2b = [pc.tile([128, 1024], b16, tag=f"x2b{d}", bufs=1,
                           name=f"x2b{d}") for d in range(ND)]
            ht = [pc.tile([128, 1024], b16, tag=f"ht{d}", bufs=1,
                          name=f"ht{d}") for d in range(ND)]

            for oc in range(ND):
                for s2 in range(2):
                    cps = psum()
                    for d in range(ND):
                        nc.tensor.matmul(
                            cps[:],
                            wvl_t[d][:, oc * 128:(oc + 1) * 128],
                            attn[d][:, s2 * 512:(s2 + 1) * 512],
                            start=(d == 0), stop=(d == ND - 1))
                    xqt = sp.tile([128, 512], f32, tag="xqt", bufs=4,
                                  name=f"xqt{oc}_{s2}")
                    nc.sync.dma_start(
                        xqt[:],
                        xq_d.ap()[oc * 128:(oc + 1) * 128,
                                  s2 * 512:(s2 + 1) * 512])
                    cc = slice(s2 * 512, (s2 + 1) * 512)
                    nc.vector.tensor_add(x2f[oc][:, cc], cps[:], xqt[:])
                    nc.vector.tensor_copy(x2b[oc][:, cc], x2f[oc][:, cc])

            for fc in range(ND):
                for s2 in range(2):
                    cps = psum()
                    for d in range(ND):
                        nc.tensor.matmul(
                            cps[:],
                            wff1_t[d][:, fc * 128:(fc + 1) * 128],
                            x2b[d][:, s2 * 512:(s2 + 1) * 512],
                            start=(d == 0), stop=(d == ND - 1))
                    cc = slice(s2 * 512, (s2 + 1) * 512)
                    nc.scalar.activation(ht[fc][:, cc], cps[:], AF.Relu,
                                         bias=bf1_t[:, fc:fc + 1])

            for oc in range(ND):
                for s2 in range(2):
                    cps = psum()
                    for f in range(ND):
                        nc.tensor.matmul(
                            cps[:],
                            wff2_t[f][:, oc * 128:(oc + 1) * 128],
                            ht[f][:, s2 * 512:(s2 + 1) * 512],
                            start=(f == 0), stop=(f == ND - 1))
                    cc = slice(s2 * 512, (s2 + 1) * 512)
                    ot = sp.tile([128, 512], f32, tag="ot", bufs=4,
                                 name=f"ot{oc}_{s2}")
                    nc.vector.tensor_add(ot[:], cps[:], x2f[oc][:, cc])
                    nc.sync.dma_start(
                        outT_d.ap()[oc * 128:(oc + 1) * 128, cc], ot[:])

    nc.compile()
    return nc


def _get_program():
    global _COMPILED
    if _COMPILED is None:
        _COMPILED = _build_program()
    return _COMPILED


def kernel(x, wqkv, w_lin, b_lin, w_ff1, b_ff1, w_ff2, b_ff2):
    from concourse.bass_utils import run_bass_kernel_spmd

    x = np.asarray(x, np.float32)
    wqkv = np.asarray(wqkv, np.float32)
    Wq = wqkv[:, :D].astype(np.float64)
    Wk = wqkv[:, D:2 * D].astype(np.float64)
    Wv = wqkv[:, 2 * D:].astype(np.float64)

    wzq = ((Wq @ Wk.T) / 2.0).astype(BF16)      # [a, d] natural layout
    wvl = (Wv @ np.asarray(w_lin, np.float64)).astype(BF16)  # Wv @ w_lin
    wff1 = np.asarray(w_ff1, np.float32).astype(BF16)
    wff2 = np.asarray(w_ff2, np.float32).astype(BF16)
    tri = (np.arange(128)[:, None] >= np.arange(128)[None, :]).astype(BF16)

    in_maps = []
    qcols_by_parity = {
        0: np.r_[0:512, 1536:2048],
        1: np.r_[512:1536],
    }
    b_lin = np.asarray(b_lin, np.float32)
    b_ff1 = np.asarray(b_ff1, np.float32)
    b_ff2 = np.asarray(b_ff2, np.float32)
    bf1 = np.ascontiguousarray(b_ff1.reshape(ND, 128))
    for c in range(NCORES):
        b, h = c // 2, c % 2
        xT32 = np.ascontiguousarray(x[b].T)               # [D, S] f32
        qcols = qcols_by_parity[h]
        qxT32 = np.ascontiguousarray(xT32[:, qcols])      # [D, 1024]
        in_maps.append({
            "xT": xT32.astype(BF16),
            "xn": x[b].astype(BF16),                      # [S, D] natural
            "qxT": qxT32.astype(BF16),
            "xq": qxT32 + b_lin[:, None],                 # b_lin folded in
            "wzq": wzq,
            "wvl": wvl,
            "wff1": wff1,
            "wff2": wff2,
            "tri": tri,
            "bf1": bf1,
            "par": np.full((1, 1), h, np.uint32),
        })

    global _LAST_IN_MAPS
    _LAST_IN_MAPS = in_maps
    nc = _get_program()
    res = run_bass_kernel_spmd(nc, in_maps, core_ids=list(range(NCORES)))

    out = np.empty((B, S, D), np.float32)
    for c in range(NCORES):
        b, h = c // 2, c % 2
        ol = res.results[c]["outT"].T                     # [1024 s, D]
        if h == 0:
            out[b, 0:512] = ol[:512]
            out[b, 1536:2048] = ol[512:]
        else:
            out[b, 512:1536] = ol
    out += b_ff2[None, None, :]
    return out


# revision 8
# speedup vs baseline: 1.0514x; 1.0364x over previous
"""Trainium2 Bass kernel for nn_MultiHeadAttention_48395691492077.

Reference (B=4, S=2048, D=1024, single head, anti-causal triu mask):
    qkv = x @ wqkv; q,k,v = split(qkv)
    scores = triu(q @ k^T / sqrt(B));  masked softmax over keys t >= s
    x2  = softmax(scores) @ v @ w_lin + b_lin + x
    out = relu(x2 @ w_ff1 + b_ff1) @ w_ff2 + b_ff2 + x2

Sharding: 8 cores = 4 batches x 2 query-halves. Each core computes
attention + MLP for its own 1024 queries against the full 2048-key
sequence of its batch. The program is identical on all cores (SPMD);
per-core differences (which queries, which mask pattern) are carried in
the input data plus one branch on the query-parity register.

Device algebra (transposed; no on-chip transposes, no K/V projections):
    uT = wzq^T.T @ qxT            with wzq = (Wq @ Wk^T)/2  (host-fused)
    scoresT[t,s] = sum_d xT[d,t] * uT[d,s]     (keys are raw x!)
    expT = exp(scoresT); diagonal 128-strips *= lower-tri mask
    den[s] broadcast = ones[128,128].T @ expT (PE), rbs = 1/den (DVE)
    H^T[d,s] = x[t,d].T @ expT  (A@X; V projection deferred)
    attnT = H^T * rbs
    x2T = wvl.T @ attnT + (xT + b_lin)  with wvl = Wv @ w_lin (host-fused:
          A@(X@Wv)@w_lin == (A@X)@(Wv@w_lin) by associativity)
    hT = relu(w_ff1.T @ x2T + b_ff1)
    outT = w_ff2.T @ hT + x2T               (+ b_ff2 added on host)
Blocks crossing the anti-causal diagonal use exact matmul widths
(128/256/384) instead of full 512; PSUM accumulation runs widest-first
so every column is initialized by the start=True matmul.
Matmul inputs are bf16 (fp32 PSUM accumulation); residuals are fp32.
"""

import numpy as np
import ml_dtypes

B, S, D = 4, 2048, 1024
NCORES = 8
BF16 = ml_dtypes.bfloat16

NT = S // 128            # 16 t-chunks
ND = D // 128            # 8 chunks of 128 along any D-sized dim

# global query-column starts of (sb0, sb1) per parity
SB_GLOBAL = {0: (0, 1536), 1: (512, 1024)}
# t-chunks each (parity, s-block) actually needs (branch-specialized)
SB_SLOTS = {
    p: {sb: list(range(SB_GLOBAL[p][sb] // 128, NT)) for sb in (0, 1)}
    for p in (0, 1)
}


def _width(parity, sb, tc):
    """Valid column count of block (sb, tc): cols [0, w) of the 512."""
    return min(512, 128 * tc - SB_GLOBAL[parity][sb] + 128)


def _is_diag(parity, sb, tc):
    """Block whose last 128 columns lie on the anti-causal diagonal."""
    return tc - SB_GLOBAL[parity][sb] // 128 < 4


_COMPILED = None
_LAST_IN_MAPS = None


def _build_program():
    from contextlib import ExitStack
    import concourse.bacc as bacc
    import concourse.mybir as mybir
    import concourse.tile as tile

    f32 = mybir.dt.float32
    b16 = mybir.dt.bfloat16
    AF = mybir.ActivationFunctionType

    nc = bacc.Bacc("TRN2", target_bir_lowering=False, debug=False,
                   num_devices=NCORES)

    # all big inputs arrive pre-arranged on the host into the on-chip
    # [128, chunk, free] layout so every DMA is contiguous per partition
    xT_d = nc.dram_tensor("xT", [128, ND * S], b16, kind="ExternalInput")
    xn_d = nc.dram_tensor("xn", [128, NT * D], b16, kind="ExternalInput")
    qxT_d = nc.dram_tensor("qxT", [128, ND * 1024], b16, kind="ExternalInput")
    xq_d = nc.dram_tensor("xq", [D, 1024], f32, kind="ExternalInput")
    wzq_d = nc.dram_tensor("wzq", [128, ND * D], b16, kind="ExternalInput")
    wvl_d = nc.dram_tensor("wvl", [128, ND * D], b16, kind="ExternalInput")
    wff1_d = nc.dram_tensor("wff1", [128, ND * D], b16, kind="ExternalInput")
    wff2_d = nc.dram_tensor("wff2", [128, ND * D], b16, kind="ExternalInput")
    tri_d = nc.dram_tensor("tri", [128, 128], b16, kind="ExternalInput")
    par_d = nc.dram_tensor("par", [1, 1], mybir.dt.uint32, kind="ExternalInput")
    bf1_d = nc.dram_tensor("bf1", [ND, 128], f32, kind="ExternalInput")
    outT_d = nc.dram_tensor("outT", [D, 1024], f32, kind="ExternalOutput")

    with tile.TileContext(nc) as tc:
        es = ExitStack()
        with es:
            pp = es.enter_context(tc.tile_pool(name="persist", bufs=1))
            sp = es.enter_context(tc.tile_pool(name="stream", bufs=2))
            ps = es.enter_context(
                tc.tile_pool(name="ps", bufs=8, space="PSUM"))
            esB = es.enter_context(ExitStack())
            pb = esB.enter_context(tc.tile_pool(name="pB", bufs=1))
            esA = ExitStack()
            pa = esA.enter_context(tc.tile_pool(name="pA", bufs=1,
                                                side="right"))

            def psum():
                t = ps.tile([128, 512], f32, tag="mm", bufs=8, name="mmps")
                return t

            # ---- constants ----
            ones_sq = pp.tile([128, 128], b16, tag="ones_sq", bufs=1)
            nc.vector.memset(ones_sq[:], 1.0)
            tri_t = pp.tile([128, 128], b16, tag="tri", bufs=1)
            # warm the PE HAM clock-gate while the first input DMAs land
            wups = psum()
            for i in range(16):
                nc.tensor.matmul(wups[:, 0:128], ones_sq[:], ones_sq[:],
                                 start=(i == 0), stop=(i == 15))

            # ---- input loads (arrival-ordered for phase-A pipelining) ----
            def chunks(dram, c0, c1, width):
                return dram.ap()[:, c0 * width:c1 * width].rearrange(
                    "p (c n) -> p c n", n=width)

            wzq_a = pa.tile([128, ND, D], b16, tag="wzq", bufs=1)
            qx_a = pa.tile([128, ND, 1024], b16, tag="qx", bufs=1)
            # a-pair granules so the a-outer phase-A loop can start early
            for a0 in range(0, ND, 2):
                nc.sync.dma_start(wzq_a[:, a0:a0 + 2],
                                  chunks(wzq_d, a0, a0 + 2, D))
                nc.sync.dma_start(qx_a[:, a0:a0 + 2],
                                  chunks(qxT_d, a0, a0 + 2, 1024))
            # xT feeds the scores pass (~45us in): one full-bandwidth DMA
            xt_a = pb.tile([128, ND, S], b16, tag="xt", bufs=1)
            nc.sync.dma_start(xt_a[:], chunks(xT_d, 0, ND, S))
            # x natural layout [t, d] feeds the A@X pass (later still)
            xn_a = pb.tile([128, NT, D], b16, tag="xn", bufs=1)
            nc.sync.dma_start(xn_a[:], chunks(xn_d, 0, NT, D))
            nc.sync.dma_start(tri_t[:], tri_d.ap())
            # b_ff1 laid out [128, ND]: bias column fc serves f-chunk fc
            bf1_t = pp.tile([128, ND], f32, tag="bf1", bufs=1)
            nc.sync.dma_start(bf1_t[:], bf1_d.ap().rearrange("c p -> p c"))
            wzq_t = [wzq_a[:, d] for d in range(ND)]
            xt = [xt_a[:, d] for d in range(ND)]
            xn = [xn_a[:, t] for t in range(NT)]
            qx = [qx_a[:, d] for d in range(ND)]

            # ---- phase A: uT[d, s] = sum_a wzq[a,d] * qxT[a,s] ----
            # a-outer in two m-halves (8 PSUM banks each) so compute starts
            # as soon as the first a-chunks of wzq/qx land.
            ut = [pb.tile([128, 1024], b16, tag=f"ut{m}", bufs=1,
                          name=f"ut{m}") for m in range(ND)]
            for half in range(2):
                ms = range(half * 4, half * 4 + 4)
                ups = {(m, sb): psum() for m in ms for sb in range(2)}
                for a in range(ND):
                    for m in ms:
                        for sb in range(2):
                            nc.tensor.matmul(
                                ups[(m, sb)][:],
                                wzq_t[a][:, m * 128:(m + 1) * 128],
                                qx[a][:, sb * 512:(sb + 1) * 512],
                                start=(a == 0), stop=(a == ND - 1))
                for m in ms:
                    for sb in range(2):
                        nc.vector.tensor_copy(
                            ut[m][:, sb * 512:(sb + 1) * 512],
                            ups[(m, sb)][:])

            # ---- free phase-A inputs; right pool for attn + phase-C weights
            esA.close()
            pr = es.enter_context(tc.tile_pool(name="pAC", bufs=1,
                                               side="right"))
            wl_a = pr.tile([128, ND, D], b16, tag="wl", bufs=1)
            nc.sync.dma_start(wl_a[:], chunks(wvl_d, 0, ND, D))
            wf1_a = pr.tile([128, ND, D], b16, tag="wf1", bufs=1)
            nc.sync.dma_start(wf1_a[:], chunks(wff1_d, 0, ND, D))
            wf2_a = pr.tile([128, ND, D], b16, tag="wf2", bufs=1)
            nc.sync.dma_start(wf2_a[:], chunks(wff2_d, 0, ND, D))
            wvl_t = [wl_a[:, d] for d in range(ND)]
            wff1_t = [wf1_a[:, d] for d in range(ND)]
            wff2_t = [wf2_a[:, d] for d in range(ND)]

            attn = [pr.tile([128, 1024], b16, tag=f"at{d}", bufs=1,
                            name=f"at{d}") for d in range(ND)]

            def phase_b(parity):
                sb_slots = SB_SLOTS[parity]
                # pass 1: scoresT -> exp -> diag mask, tc-outer
                et = {}
                for tcn in range(NT):
                    work = [(sb, _width(parity, sb, tcn))
                            for sb in (0, 1) if tcn in sb_slots[sb]]
                    scp = {sb: psum() for sb, _ in work}
                    for d in range(ND):
                        for sb, w in work:
                            nc.tensor.matmul(
                                scp[sb][:, 0:w],
                                xt[d][:, tcn * 128:(tcn + 1) * 128],
                                ut[d][:, sb * 512:sb * 512 + w],
                                start=(d == 0), stop=(d == ND - 1))
                    for sb, w in work:
                        e = pb.tile([128, w], b16, tag=f"et{sb}_{tcn}",
                                    bufs=1, name=f"et{parity}_{sb}_{tcn}")
                        et[(sb, tcn)] = e
                        nc.scalar.activation(e[:], scp[sb][:, 0:w], AF.Exp)
                        if _is_diag(parity, sb, tcn):
                            nc.vector.tensor_mul(
                                e[:, w - 128:w], e[:, w - 128:w], tri_t[:])

                # pass 2: den (broadcast), recip, A@X, normalize.
                # Accumulate widest-first (descending tc) so the start=True
                # matmul initializes the full 512 columns.
                rbs = {}
                for sb in (0, 1):
                    slots = sb_slots[sb][::-1]
                    den_ps = psum()
                    for k, tcn in enumerate(slots):
                        w = _width(parity, sb, tcn)
                        nc.tensor.matmul(
                            den_ps[:, 0:w], ones_sq[:], et[(sb, tcn)][:],
                            start=(k == 0), stop=(k == len(slots) - 1))
                    r = sp.tile([128, 512], f32, tag="rbs", bufs=2,
                                name=f"rbs{parity}_{sb}")
                    nc.vector.reciprocal(r[:], den_ps[:])
                    rbs[sb] = r

                for dc in range(ND):
                    axp = {sb: psum() for sb in (0, 1)}
                    for sb in (0, 1):
                        slots = sb_slots[sb][::-1]
                        for k, tcn in enumerate(slots):
                            w = _width(parity, sb, tcn)
                            nc.tensor.matmul(
                                axp[sb][:, 0:w],
                                xn[tcn][:, dc * 128:(dc + 1) * 128],
                                et[(sb, tcn)][:],
                                start=(k == 0),
                                stop=(k == len(slots) - 1))
                    for sb in (0, 1):
                        nc.vector.tensor_mul(
                            attn[dc][:, sb * 512:(sb + 1) * 512],
                            axp[sb][:], rbs[sb][:])

            par_regs = nc.alloc_registers("par_regs")
            nc.regs_load(par_regs, par_d.ap()[0:1, 0:1])
            par = nc.snap(par_regs, donate=True, min_val=0, max_val=1)
            with tc.If(par < 1) as cmp:
                phase_b(0)
            with cmp.Else():
                phase_b(1)

            # ---- free pB (ut/xt/xn/et); left pool for phase-C tiles ----
            esB.close()
            esC = es.enter_context(ExitStack())
            pc = esC.enter_context(tc.tile_pool(name="pC", bufs=1))

            x2f = [pc.tile([128, 1024], f32, tag=f"x2f{d}", bufs=1,
                           name=f"x2f{d}") for d in range(ND)]
            x2b = [pc.tile([128, 1024], b16, tag=f"x2b{d}", bufs=1,
                           name=f"x2b{d}") for d in range(ND)]
            ht = [pc.tile([128, 1024], b16, tag=f"ht{d}", bufs=1,
                          name=f"ht{d}") for d in range(ND)]

            for oc in range(ND):
                for s2 in range(2):
                    cps = psum()
                    for d in range(ND):
                        nc.tensor.matmul(
                            cps[:],
                            wvl_t[d][:, oc * 128:(oc + 1) * 128],
                            attn[d][:, s2 * 512:(s2 + 1) * 512],
                            start=(d == 0), stop=(d == ND - 1))
                    xqt = sp.tile([128, 512], f32, tag="xqt", bufs=4,
                                  name=f"xqt{oc}_{s2}")
                    nc.sync.dma_start(
                        xqt[:],
                        xq_d.ap()[oc * 128:(oc + 1) * 128,
                                  s2 * 512:(s2 + 1) * 512])
                    cc = slice(s2 * 512, (s2 + 1) * 512)
                    nc.vector.tensor_add(x2f[oc][:, cc], cps[:], xqt[:])
                    nc.vector.tensor_copy(x2b[oc][:, cc], x2f[oc][:, cc])

            for fc in range(ND):
                for s2 in range(2):
                    cps = psum()
                    for d in range(ND):
                        nc.tensor.matmul(
                            cps[:],
                            wff1_t[d][:, fc * 128:(fc + 1) * 128],
                            x2b[d][:, s2 * 512:(s2 + 1) * 512],
                            start=(d == 0), stop=(d == ND - 1))
                    cc = slice(s2 * 512, (s2 + 1) * 512)
                    nc.scalar.activation(ht[fc][:, cc], cps[:], AF.Relu,
                                         bias=bf1_t[:, fc:fc + 1])

            for oc in range(ND):
                for s2 in range(2):
                    cps = psum()
                    for f in range(ND):
                        nc.tensor.matmul(
                            cps[:],
                            wff2_t[f][:, oc * 128:(oc + 1) * 128],
                            ht[f][:, s2 * 512:(s2 + 1) * 512],
                            start=(f == 0), stop=(f == ND - 1))
                    cc = slice(s2 * 512, (s2 + 1) * 512)
                    ot = sp.tile([128, 512], f32, tag="ot", bufs=4,
                                 name=f"ot{oc}_{s2}")
                    nc.vector.tensor_add(ot[:], cps[:], x2f[oc][:, cc])
                    nc.sync.dma_start(
                        outT_d.ap()[oc * 128:(oc + 1) * 128, cc], ot[:])

    nc.compile()
    return nc


def _get_program():
    global _COMPILED
    if _COMPILED is None:
        _COMPILED = _build_program()
    return _COMPILED


def _p128(arr):
    """[c*128, C] -> [128, c*C]: the on-chip chunked layout, so device DMAs
    are contiguous per partition."""
    c = arr.shape[0] // 128
    return np.ascontiguousarray(
        arr.reshape(c, 128, -1).transpose(1, 0, 2).reshape(128, -1))


def kernel(x, wqkv, w_lin, b_lin, w_ff1, b_ff1, w_ff2, b_ff2):
    from concourse.bass_utils import run_bass_kernel_spmd

    x = np.asarray(x, np.float32)
    wqkv = np.asarray(wqkv, np.float32)
    Wq = wqkv[:, :D].astype(np.float64)
    Wk = wqkv[:, D:2 * D].astype(np.float64)
    Wv = wqkv[:, 2 * D:].astype(np.float64)

    wzq = _p128(((Wq @ Wk.T) / 2.0).astype(BF16))   # [a, d] natural layout
    wvl = _p128((Wv @ np.asarray(w_lin, np.float64)).astype(BF16))
    wff1 = _p128(np.asarray(w_ff1, np.float32).astype(BF16))
    wff2 = _p128(np.asarray(w_ff2, np.float32).astype(BF16))
    tri = (np.arange(128)[:, None] >= np.arange(128)[None, :]).astype(BF16)

    in_maps = []
    qcols_by_parity = {
        0: np.r_[0:512, 1536:2048],
        1: np.r_[512:1536],
    }
    b_lin = np.asarray(b_lin, np.float32)
    b_ff1 = np.asarray(b_ff1, np.float32)
    b_ff2 = np.asarray(b_ff2, np.float32)
    bf1 = np.ascontiguousarray(b_ff1.reshape(ND, 128))
    for c in range(NCORES):
        b, h = c // 2, c % 2
        xT32 = np.ascontiguousarray(x[b].T)               # [D, S] f32
        qcols = qcols_by_parity[h]
        qxT32 = np.ascontiguousarray(xT32[:, qcols])      # [D, 1024]
        in_maps.append({
            "xT": _p128(xT32.astype(BF16)),
            "xn": _p128(x[b].astype(BF16)),               # [S, D] natural
            "qxT": _p128(qxT32.astype(BF16)),
            "xq": qxT32 + b_lin[:, None],                 # b_lin folded in
            "wzq": wzq,
            "wvl": wvl,
            "wff1": wff1,
            "wff2": wff2,
            "tri": tri,
            "bf1": bf1,
            "par": np.full((1, 1), h, np.uint32),
        })

    global _LAST_IN_MAPS
    _LAST_IN_MAPS = in_maps
    nc = _get_program()
    res = run_bass_kernel_spmd(nc, in_maps, core_ids=list(range(NCORES)))

    out = np.empty((B, S, D), np.float32)
    for c in range(NCORES):
        b, h = c // 2, c % 2
        ol = res.results[c]["outT"].T                     # [1024 s, D]
        if h == 0:
            out[b, 0:512] = ol[:512]
            out[b, 1536:2048] = ol[512:]
        else:
            out[b, 512:1536] = ol
    out += b_ff2[None, None, :]
    return out
